# revision 1
# baseline (speedup 1.0000x reference)
"""ABCAttention (fla chunk_abc semantics) on 8 TRN2 NeuronCores.

Sharding: core c -> (batch b = c//2, head-pair hg = c%2). Each core computes
its batch's projections for its 2 heads (512 of 1024 feature columns), runs
the chunked ABC scan for those 2 (b,h) pairs, applies the gated-RMSNorm
epilogue, and produces a partial o_w product (its 512 rows of o_w). The host
sums the two partials per batch and transposes (device emits out^T).

Math: the ABC scan is rewritten in chunked form (C=128) with UNNORMALIZED
slot state:
    W_t[m]  = cumsum_t exp(s)          (running normalizer; z = ln W)
    ok_t    = (q_t @ Uk + A_masked @ E) / W_t,  Uk = sum_j k_j E_j^T
    qw_t    = softmax_m(ok_t) / W_t
    ov_t    = qw_t @ Uv + (E @ qw^T)^T_masked @ V,  Uv = sum_j E_j v_j^T
This matches jax cumlogsumexp/scan reference to ~1e-6 (validated in fp32
numpy); fp32r matmuls add ~1e-4 noise.
"""
import sys
import numpy as np
import ml_dtypes

sys.path.insert(0, '/opt/trn_rl_repo')

import concourse.bass as bass        # noqa: E402
import concourse.bacc as bacc        # noqa: E402
import concourse.mybir as mybir      # noqa: E402
import concourse.tile as tile        # noqa: E402

f32 = mybir.dt.float32
f32r = mybir.dt.float32r
bf16 = mybir.dt.bfloat16
AF = mybir.ActivationFunctionType
ALU = mybir.AluOpType
AX = mybir.AxisListType

B, T, D, H = 4, 2048, 1024, 4
DK = DV = M = 256
HALF = 128
GATE_NORM = 16.0
EPS = 1e-5
ROPE_BASE = 10000.0
QSCALE = 1.0 / 16.0          # DK ** -0.5

NCORE = 8
COLS = 512                   # feature columns per core (2 heads)
SLAB = 512                   # tokens per pipeline slab
NSLAB = T // SLAB
C = 128                      # scan chunk length
CPS = SLAB // C              # chunks per slab
KB = D // 128                # 8 contraction blocks


def ts(i, n=128):
    return bass.ts(i, n)


def _patch_ldw_opt():
    """walrus --enable-ldw-opt=false forces foreground weight-buffer loads,
    serializing LDWEIGHTS with MATMUL. Flip it to true."""
    import concourse.bass_utils as bu
    if getattr(bu, "_abc_ldw_patch", False):
        return
    orig = bu.run_command

    def patched(argv, **kw):
        argv = ["--enable-ldw-opt=true" if a == "--enable-ldw-opt=false" else a
                for a in argv]
        return orig(argv, **kw)

    bu.run_command = patched
    bu._abc_ldw_patch = True


def _patch_act_tables():
    """Keep only natural_log_exp_and_others selectable (ids preserved) so the
    table-load pass stops thrashing exp_and_others <-> natural_log."""
    import concourse.hw_specs as hw_specs
    if getattr(bacc, "_abc_act_patch", False):
        return
    orig = hw_specs.get_activation_tables

    def patched(module_arch):
        tabs = orig(module_arch)
        keep = "natural_log_exp_and_others"
        return {k: (v if k == keep else set()) for k, v in tabs.items()}

    bacc.get_activation_tables = patched
    bacc._abc_act_patch = True


def build():
    _patch_act_tables()
    nc = bacc.Bacc(None, target_bir_lowering=False)

    xT_e = nc.declare_dram_parameter("xT", [D, T], bf16, isOutput=False)
    w_e = {}
    for nm in ("wq", "wk", "ws", "wv", "wg"):
        w_e[nm] = nc.declare_dram_parameter(nm, [D, COLS], bf16, isOutput=False)
    wsg_e = nc.declare_dram_parameter("wsg", [D, 2], bf16, isOutput=False)
    wo_e = nc.declare_dram_parameter("wo", [COLS, D], bf16, isOutput=False)
    cos_e = nc.declare_dram_parameter("cosT", [HALF, T], f32, isOutput=False)
    sin_e = nc.declare_dram_parameter("sinT", [HALF, T], f32, isOutput=False)
    trq_e = nc.declare_dram_parameter("trilq", [128, 128], f32, isOutput=False)
    tr1_e = nc.declare_dram_parameter("tril1", [128, 128], f32, isOutput=False)
    tr1b_e = nc.declare_dram_parameter("tril1b", [128, 128], bf16, isOutput=False)
    idn_e = nc.declare_dram_parameter("ident", [128, 128], f32, isOutput=False)
    idnb_e = nc.declare_dram_parameter("identb", [128, 128], bf16, isOutput=False)
    one_e = nc.declare_dram_parameter("onesc", [1, 128], bf16, isOutput=False)
    onk_e = nc.declare_dram_parameter("onek", [128, 1], bf16, isOutput=False)
    zr_e = nc.declare_dram_parameter("zeros", [128, 512], bf16, isOutput=False)
    out_e = nc.declare_dram_parameter("outT", [D, T], f32, isOutput=True)

    with tile.TileContext(nc) as tc:
        with (tc.tile_pool(name="weights", bufs=1) as wp,
              tc.tile_pool(name="consts", bufs=1) as cp,
              tc.tile_pool(name="slab", bufs=2) as sp,
              tc.tile_pool(name="scan", bufs=2) as kp,
              tc.tile_pool(name="psA", bufs=3, space="PSUM") as pa,
              tc.tile_pool(name="psB", bufs=4, space="PSUM") as pb,
              tc.tile_pool(name="psW", bufs=1, space="PSUM") as pw):
            # ---- resident constants & weights -------------------------------
            wt = {}
            for nm in ("wq", "wk", "ws", "wv", "wg"):
                wt[nm] = [wp.tile([128, COLS], bf16, tag=f"{nm}{kb}", name=f"{nm}{kb}")
                          for kb in range(KB)]
            wsg_t = [wp.tile([128, 2], bf16, tag=f"wsg{kb}", name=f"wsg{kb}") for kb in range(KB)]
            wo_t = [wp.tile([128, D], bf16, tag=f"wo{q}", name=f"wo{q}") for q in range(4)]
            # critical-path weights first (sg chain + q/k projections);
            # spread issue across engine queues to parallelize descriptor setup
            engs = [nc.sync, nc.scalar, nc.gpsimd]
            for kb in range(KB):
                engs[kb % 3].dma_start(wsg_t[kb][:], wsg_e[ts(kb), :])
            def _weight_dmas():
                i = 0
                for nm in ("wq", "wk", "ws", "wv", "wg"):
                    for kb in range(KB):
                        engs[i % 3].dma_start(wt[nm][kb][:], w_e[nm][ts(kb), :])
                        i += 1
                for q in range(4):
                    engs[q % 3].dma_start(wo_t[q][:], wo_e[ts(q), :])
            trilq = cp.tile([128, 128], f32, tag="trilq", name="trilq")
            tril1 = cp.tile([128, 128], f32, tag="tril1", name="tril1")
            tril1r = cp.tile([128, 128], bf16, tag="tril1r", name="tril1r")
            identf = cp.tile([128, 128], f32, tag="identf", name="identf")
            identr = cp.tile([128, 128], bf16, tag="identr", name="identr")
            onescr = cp.tile([1, 128], bf16, tag="onescr", name="onescr")
            nc.sync.dma_start(trilq[:], trq_e[:])
            nc.sync.dma_start(tril1[:], tr1_e[:])
            nc.sync.dma_start(tril1r[:], tr1b_e[:])
            nc.sync.dma_start(identf[:], idn_e[:])
            nc.sync.dma_start(identr[:], idnb_e[:])
            nc.sync.dma_start(onescr[:], one_e[:])
            onekr = cp.tile([128, 1], bf16, tag="onekr", name="onekr")
            nc.sync.dma_start(onekr[:], onk_e[:])
            zeros2 = cp.tile([2, SLAB], f32, tag="zeros2", name="zeros2")
            nc.vector.memset(zeros2[:], 0.0)

            # ---- persistent scan state (ping-pong SBUF tiles) ---------------
            uk_cur, uv_cur = [], []
            for h in range(2):
                uk = kp.tile([128, 2 * M], bf16, tag=f"uk{h}", name=f"uk{h}", bufs=2)
                uv = kp.tile([128, 2 * DV], bf16, tag=f"uv{h}", name=f"uv{h}", bufs=2)
                nc.sync.dma_start(uk[:], zr_e[:])
                nc.sync.dma_start(uv[:], zr_e[:])
                uk_cur.append(uk)
                uv_cur.append(uv)
            wlastf = kp.tile([1, COLS], f32, tag="wlastf", name="wlastf", bufs=2)
            nc.vector.memset(wlastf[:], 0.0)
            wlast = kp.tile([1, COLS], bf16, tag="wlastb", name="wlastb", bufs=5)
            nc.sync.dma_start(wlast[:], zr_e[0:1, :])

            sg_carry = kp.tile([2, 1], f32, tag="sgc", name="sgc", bufs=2)
            nc.vector.memset(sg_carry[:], 0.0)

            def load_slab(s):
                tok = slice(s * SLAB, (s + 1) * SLAB)
                xs = [sp.tile([128, SLAB], bf16, tag=f"xs{kb}", name=f"xs{kb}", bufs=2)
                      for kb in range(KB)]
                engs2 = [nc.sync, nc.scalar, nc.gpsimd]
                for kb in range(KB):
                    engs2[kb % 3].dma_start(xs[kb][:], xT_e[ts(kb), tok])
                return xs

            def sg_chain(xs):
                nonlocal sg_carry
                ps_sg = pb.tile([128, SLAB], f32, tag="psB", name="psB")
                for kb in range(KB):
                    nc.tensor.matmul(ps_sg[0:2, :], wsg_t[kb][:], xs[kb][:],
                                     start=(kb == 0), stop=(kb == KB - 1))
                e_sg = kp.tile([2, SLAB], f32, tag="esg", name="esg", bufs=1)
                nc.scalar.activation(e_sg[:], ps_sg[0:2, :], AF.Exp, scale=-1.0)
                u_sg = kp.tile([2, SLAB], f32, tag="usg", name="usg", bufs=1)
                nc.vector.tensor_scalar_add(u_sg[:], e_sg[:], 1.0)
                l_sg = kp.tile([2, SLAB], f32, tag="lsg", name="lsg", bufs=1)
                nc.scalar.activation(l_sg[:], u_sg[:], AF.Ln)
                cum = kp.tile([2, SLAB], f32, tag="cum", name="cum", bufs=1)
                nc.vector.tensor_tensor_scan(cum[:], l_sg[:], zeros2[:],
                                             sg_carry[:], ALU.add, ALU.add)
                sg_carry = kp.tile([2, 1], f32, tag="sgc", name="sgc", bufs=2)
                nc.scalar.copy(sg_carry[:], cum[:, SLAB - 1:SLAB])
                lam = kp.tile([2, SLAB], bf16, tag="lam", name="lam", bufs=1)
                nc.scalar.activation(lam[:], cum[:], AF.Exp,
                                     scale=-1.0 / GATE_NORM)
                lam1 = kp.tile([1, SLAB], bf16, tag="lam1", name="lam1", bufs=1)
                nc.sync.dma_start(lam1[:], lam[1:2, :])
                lam_bc = []
                for h in range(2):
                    bcst = kp.tile([128, SLAB], bf16, tag=f"lamb{h}", name=f"lamb{h}")
                    nc.gpsimd.partition_broadcast(
                        bcst[:], lam[0:1, :] if h == 0 else lam1[:])
                    lam_bc.append(bcst)
                return lam_bc

            def load_trig(s):
                tok2 = slice(s * SLAB, (s + 1) * SLAB)
                c_t = sp.tile([HALF, SLAB], f32, tag="cos_sl", name="cos_sl", bufs=2)
                s_t = sp.tile([HALF, SLAB], f32, tag="sin_sl", name="sin_sl", bufs=2)
                nc.sync.dma_start(c_t[:], cos_e[:, tok2])
                nc.sync.dma_start(s_t[:], sin_e[:, tok2])
                return c_t, s_t

            xs = load_slab(0)
            trig = load_trig(0)
            _weight_dmas()
            lam_bc = sg_chain(xs)
            for s in range(NSLAB):
                tok = slice(s * SLAB, (s + 1) * SLAB)
                cos_sl, sin_sl = trig

                # ---- q/k projections (feature-major) + rope ----------------
                qT, kT = [], []           # 4 col-tiles each: [128, SLAB] f32r
                for nm, dest, gated in (("wq", qT, False), ("wk", kT, True)):
                    for h in range(2):
                        ps1 = pa.tile([128, SLAB], f32, tag="psA", name="psA")
                        ps2 = pa.tile([128, SLAB], f32, tag="psA", name="psA")
                        for kb in range(KB):
                            nc.tensor.matmul(
                                ps1[:], wt[nm][kb][:, ts(2 * h)], xs[kb][:],
                                start=(kb == 0), stop=(kb == KB - 1))
                        for kb in range(KB):
                            nc.tensor.matmul(
                                ps2[:], wt[nm][kb][:, ts(2 * h + 1)], xs[kb][:],
                                start=(kb == 0), stop=(kb == KB - 1))
                        cs = cos_sl[:]
                        sn = sin_sl[:]
                        m1 = kp.tile([128, SLAB], bf16, tag="ropeA", name="ropeA", bufs=1)
                        m2 = kp.tile([128, SLAB], bf16, tag="ropeB", name="ropeB", bufs=1)
                        o1 = kp.tile([128, SLAB], bf16, tag=f"{nm}o{2*h}", name=f"{nm}o{2*h}")
                        nc.vector.tensor_tensor(m1[:], ps1[:], cs, ALU.mult)
                        nc.vector.tensor_tensor(m2[:], ps2[:], sn, ALU.mult)
                        m3 = kp.tile([128, SLAB], bf16, tag="ropeC", name="ropeC", bufs=1)
                        m4 = kp.tile([128, SLAB], bf16, tag="ropeD", name="ropeD", bufs=1)
                        o2 = kp.tile([128, SLAB], bf16, tag=f"{nm}o{2*h+1}", name=f"{nm}o{2*h+1}")
                        nc.vector.tensor_tensor(m3[:], ps2[:], cs, ALU.mult)
                        nc.vector.tensor_tensor(m4[:], ps1[:], sn, ALU.mult)
                        if not gated:
                            nc.vector.tensor_tensor(o1[:], m1[:], m2[:],
                                                    ALU.subtract)
                            nc.vector.tensor_tensor(o2[:], m3[:], m4[:],
                                                    ALU.add)
                        else:
                            r1 = kp.tile([128, SLAB], bf16, tag="ropeE", name="ropeE", bufs=1)
                            r2 = kp.tile([128, SLAB], bf16, tag="ropeF", name="ropeF", bufs=1)
                            nc.vector.tensor_tensor(r1[:], m1[:], m2[:],
                                                    ALU.subtract)
                            nc.vector.tensor_tensor(r2[:], m3[:], m4[:],
                                                    ALU.add)
                            nc.vector.tensor_tensor(o1[:], r1[:], lam_bc[h][:],
                                                    ALU.mult)
                            nc.vector.tensor_tensor(o2[:], r2[:], lam_bc[h][:],
                                                    ALU.mult)
                        dest.extend([o1, o2])

                # ---- s/v/g projections (token-major) -----------------------
                E_t, V_t, G_t = [], [], []
                for ci in range(CPS):
                    for nm in ("ws", "wv", "wg"):
                        ps = pa.tile([128, COLS], f32, tag="psA", name="psA")
                        for kb in range(KB):
                            nc.tensor.matmul(
                                ps[:, 0:COLS], xs[kb][:, ts(ci)], wt[nm][kb][:],
                                start=(kb == 0), stop=(kb == KB - 1))
                        if nm == "ws":
                            t = sp.tile([128, COLS], bf16, tag=f"E{ci}", name=f"E{ci}")
                            nc.scalar.activation(t[:], ps[:, 0:COLS], AF.Exp)
                            E_t.append(t)
                        elif nm == "wv":
                            t = sp.tile([128, COLS], bf16, tag=f"V{ci}", name=f"V{ci}")
                            nc.scalar.copy(t[:], ps[:, 0:COLS])
                            V_t.append(t)
                        else:
                            t = sp.tile([128, COLS], bf16, tag=f"G{ci}", name=f"G{ci}")
                            nc.scalar.copy(t[:], ps[:, 0:COLS])
                            G_t.append(t)

                # ---- per-chunk scan ----------------------------------------
                ssq = kp.tile([128, 2 * CPS], f32, tag="ssq", name="ssq")
                ov_sb = []
                # per-chunk W carries (chain hoisted off the scan path):
                # wl_b[ci] = bf16 carry INTO chunk ci = colsums of chunks < ci
                wl_b = [wlast]
                for ci in range(CPS):
                    ps_wd = pw.tile([128, COLS], f32, tag="ps_wd", name="ps_wd")
                    nc.tensor.matmul(ps_wd[0:1, :], onekr[:], E_t[ci][:],
                                     start=True, stop=True)
                    wlast_new = kp.tile([1, COLS], f32, tag="wlastf", name="wlastf", bufs=2)
                    nc.vector.tensor_tensor(wlast_new[:], ps_wd[0:1, :],
                                            wlastf[:], ALU.add)
                    wlastf = wlast_new
                    if ci < CPS - 1:
                        wb = kp.tile([1, COLS], bf16, tag="wlastb", name="wlastb", bufs=5)
                        nc.scalar.copy(wb[:], wlast_new[:])
                        wl_b.append(wb)
                wlast = kp.tile([1, COLS], bf16, tag="wlastb", name="wlastb", bufs=5)
                nc.scalar.copy(wlast[:], wlastf[:])

                rW_t = []
                for ci in range(CPS):
                    ps_w = pa.tile([128, COLS], f32, tag="psA", name="psA")
                    nc.tensor.matmul(ps_w[:, 0:COLS], tril1r[:], E_t[ci][:],
                                     start=True, stop=False)
                    nc.tensor.matmul(ps_w[:, 0:COLS], onescr[:], wl_b[ci][:],
                                     start=False, stop=True)
                    rWf = kp.tile([128, COLS], f32, tag="rWf", name="rWf", bufs=2)
                    nc.vector.reciprocal_approx_fast(rWf[:], ps_w[:, 0:COLS])
                    rW = kp.tile([128, COLS], bf16, tag="rW", name="rW", bufs=5)
                    nc.vector.tensor_copy(rW[:], rWf[:])
                    rW_t.append(rW)

                rden_t = {}
                for ci in range(CPS):
                    rW = rW_t[ci]

                    ovc = kp.tile([128, COLS], bf16, tag=f"ovc{ci}", name=f"ovc{ci}")
                    ov_sb.append(ovc)
                    for h in range(2):
                        hsl = slice(h * M, (h + 1) * M)
                        Ec = E_t[ci][:, hsl]
                        Vc = V_t[ci][:, hsl]
                        qTc = [qT[2 * h][:, ts(ci)], qT[2 * h + 1][:, ts(ci)]]
                        kTc = [kT[2 * h][:, ts(ci)], kT[2 * h + 1][:, ts(ci)]]

                        # At = mask(K^T Q)/16   (shares a psum tile with St)
                        ps_at = pb.tile([128, 256], f32, tag="psB", name="psB")
                        nc.tensor.matmul(ps_at[:, 0:128], kTc[0], qTc[0],
                                         start=True, stop=False)
                        nc.tensor.matmul(ps_at[:, 0:128], kTc[1], qTc[1],
                                         start=False, stop=True)
                        at_sb = kp.tile([128, 128], bf16, tag="at", name="at")
                        nc.vector.tensor_tensor(at_sb[:], ps_at[:, 0:128],
                                                trilq[:], ALU.mult)

                        # Em, Ktok transposes (one bf16 psum tile, 4 quads)
                        ps_tr = pb.tile([128, 512], bf16, tag="psB", name="psB")
                        for blk in range(2):
                            nc.tensor.transpose(ps_tr[:, ts(blk)],
                                                Ec[:, ts(blk)], identr[:])
                            nc.tensor.transpose(ps_tr[:, ts(2 + blk)],
                                                kTc[blk], identr[:])
                        em_sb = kp.tile([128, 2 * 128], bf16, tag="em", name="em")
                        nc.scalar.copy(em_sb[:], ps_tr[:, 0:256])
                        kt_sb = kp.tile([128, 2 * 128], bf16, tag="kt", name="kt")
                        nc.vector.tensor_copy(kt_sb[:], ps_tr[:, 256:512])

                        # ok = At^T E + q Uk  (Uk pre-scaled by 1/16)
                        ps_ok = pb.tile([128, SLAB], f32, tag="psB", name="psB")
                        ps_ov = ps_ok[:, 256:512]
                        nc.tensor.matmul(ps_ok[:, 0:M], at_sb[:], Ec,
                                         start=True, stop=False)
                        nc.tensor.matmul(ps_ok[:, 0:M], qTc[0],
                                         uk_cur[h][:, 0:M],
                                         start=False, stop=False)
                        nc.tensor.matmul(ps_ok[:, 0:M], qTc[1],
                                         uk_cur[h][:, M:2 * M],
                                         start=False, stop=True)
                        rWc = rW[:, hsl]
                        okn = kp.tile([128, M], f32, tag="okn", name="okn")
                        nc.vector.tensor_tensor(okn[:], ps_ok[:, 0:M], rWc,
                                                ALU.mult)
                        p = kp.tile([128, M], bf16, tag="p", name="p")
                        den = kp.tile([128, 1], f32, tag="den", name="den")
                        nc.scalar.activation(p[:], okn[:], AF.Exp,
                                             accum_out=den[:])
                        rden = kp.tile([128, 1], f32, tag=f"rden{ci}{h}",
                                       name="rden", bufs=2)
                        nc.vector.reciprocal_approx_fast(rden[:], den[:])
                        rden2 = kp.tile([128, 1], f32, tag="rden2", name="rden2")
                        nc.vector.tensor_tensor(rden2[:], rden[:], rden[:],
                                                ALU.mult)
                        rden_t[(ci, h)] = rden
                        qw = kp.tile([128, M], bf16, tag="qw", name="qw")
                        nc.vector.tensor_tensor(qw[:], p[:], rWc, ALU.mult)

                        # qw^T
                        ps_qt = pb.tile([128, 256], bf16, tag="psB", name="psB")
                        for blk in range(2):
                            nc.tensor.transpose(ps_qt[:, ts(blk)],
                                                qw[:, ts(blk)], identr[:])
                        qwt = kp.tile([128, 2 * 128], bf16, tag="qwt", name="qwt")
                        nc.scalar.copy(qwt[:], ps_qt[:, 0:256])

                        # St = mask(E qw^T)
                        ps_st = ps_at[:, 128:256]
                        nc.tensor.matmul(ps_st, em_sb[:, 0:128],
                                         qwt[:, 0:128], start=True, stop=False)
                        nc.tensor.matmul(ps_st, em_sb[:, 128:256],
                                         qwt[:, 128:256], start=False,
                                         stop=True)
                        st_sb = kp.tile([128, 128], bf16, tag="st", name="st")
                        nc.vector.tensor_tensor(st_sb[:], ps_st,
                                                tril1[:], ALU.mult)

                        # ov = St^T V + qw Uv   (into ps_ok's 2nd half)
                        nc.tensor.matmul(ps_ov, st_sb[:], Vc,
                                         start=True, stop=False)
                        nc.tensor.matmul(ps_ov, qwt[:, 0:128],
                                         uv_cur[h][:, 0:DV],
                                         start=False, stop=False)
                        nc.tensor.matmul(ps_ov, qwt[:, 128:256],
                                         uv_cur[h][:, DV:2 * DV],
                                         start=False, stop=True)
                        nc.scalar.copy(ovc[:, hsl], ps_ov)
                        nc.vector.scalar_tensor_tensor(
                            okn[:], ovc[:, hsl], rden2[:], ovc[:, hsl],
                            ALU.mult, ALU.mult,
                            accum_out=ssq[:, 2 * ci + h:2 * ci + h + 1])

                        # state updates
                        ps_dk = pa.tile([128, 2 * M], f32, tag="psA", name="psA")
                        nc.tensor.matmul(ps_dk[:, 0:M], kt_sb[:, 0:128], Ec,
                                         start=True, stop=True)
                        nc.tensor.matmul(ps_dk[:, M:2 * M], kt_sb[:, 128:256],
                                         Ec, start=True, stop=True)
                        uk_new = kp.tile([128, 2 * M], bf16, tag=f"uk{h}", name=f"uk{h}", bufs=2)
                        nc.vector.scalar_tensor_tensor(
                            uk_new[:], ps_dk[:], QSCALE, uk_cur[h][:],
                            ALU.mult, ALU.add)
                        uk_cur[h] = uk_new
                        ps_dv = pa.tile([128, 2 * DV], f32, tag="psA", name="psA")
                        nc.tensor.matmul(ps_dv[:, 0:DV], Ec[:, 0:128], Vc,
                                         start=True, stop=True)
                        nc.tensor.matmul(ps_dv[:, DV:2 * DV], Ec[:, 128:256],
                                         Vc, start=True, stop=True)
                        uv_new = kp.tile([128, 2 * DV], bf16, tag=f"uv{h}", name=f"uv{h}", bufs=2)
                        nc.vector.scalar_tensor_tensor(
                            uv_new[:], ps_dv[:], 1.0, uv_cur[h][:],
                            ALU.mult, ALU.add)
                        uv_cur[h] = uv_new

                # ---- epilogue: rsqrt cols, swish gate, og, out proj --------
                vv = kp.tile([128, 2 * CPS], f32, tag="vv", name="vv")
                nc.vector.tensor_scalar(vv[:], ssq[:], 1.0 / DV, EPS,
                                        ALU.mult, ALU.add)
                lnv = kp.tile([128, 2 * CPS], f32, tag="lnv", name="lnv")
                nc.scalar.activation(lnv[:], vv[:], AF.Ln)
                rsq = kp.tile([128, 2 * CPS], f32, tag="rsq", name="rsq")
                nc.scalar.activation(rsq[:], lnv[:], AF.Exp, scale=-0.5)

                ogt = [sp.tile([128, SLAB], bf16, tag=f"ogt{q}", name=f"ogt{q}")
                       for q in range(4)]
                for ci in range(CPS):
                    gch = G_t[ci]
                    eneg = kp.tile([128, COLS], bf16, tag="eneg", name="eneg")
                    nc.scalar.activation(eneg[:], gch[:], AF.Exp, scale=-1.0)
                    u1 = kp.tile([128, COLS], bf16, tag="u1", name="u1")
                    nc.vector.tensor_scalar_add(u1[:], eneg[:], 1.0)
                    lnu = kp.tile([128, COLS], bf16, tag="lnu", name="lnu")
                    nc.scalar.activation(lnu[:], u1[:], AF.Ln)
                    sig = kp.tile([128, COLS], bf16, tag="sig", name="sig")
                    nc.scalar.activation(sig[:], lnu[:], AF.Exp, scale=-1.0)
                    p1 = kp.tile([128, COLS], bf16, tag="p1", name="p1")
                    nc.vector.tensor_tensor(p1[:], ov_sb[ci][:], gch[:],
                                            ALU.mult)
                    p2 = kp.tile([128, COLS], bf16, tag="p2", name="p2")
                    nc.vector.tensor_tensor(p2[:], p1[:], sig[:], ALU.mult)
                    og = kp.tile([128, COLS], bf16, tag="og", name="og")
                    for h in range(2):
                        hsl = slice(h * M, (h + 1) * M)
                        rr = kp.tile([128, 1], f32, tag="rr", name="rr")
                        nc.vector.tensor_tensor(
                            rr[:], rsq[:, 2 * ci + h:2 * ci + h + 1],
                            rden_t[(ci, h)][:], ALU.mult)
                        nc.vector.tensor_scalar(
                            og[:, hsl], p2[:, hsl], rr[:], None, ALU.mult)
                    for hf in range(2):
                        ps_og = pb.tile([128, 256], bf16, tag="psB", name="psB")
                        for qq in range(2):
                            q = 2 * hf + qq
                            nc.tensor.transpose(ps_og[:, ts(qq)],
                                                og[:, ts(q)], identr[:])
                            nc.scalar.copy(ogt[q][:, ci * 128:ci * 128 + 128],
                                           ps_og[:, ts(qq)])
                if s + 1 < NSLAB:
                    xs_next = load_slab(s + 1)
                    trig_next = load_trig(s + 1)
                    lam_bc_next = sg_chain(xs_next)
                else:
                    xs_next, lam_bc_next, trig_next = None, None, None
                for ct in range(8):
                    ps_o = pa.tile([128, SLAB], f32, tag="psA", name="psA")
                    for q in range(4):
                        nc.tensor.matmul(ps_o[:], wo_t[q][:, ts(ct)],
                                         ogt[q][:], start=(q == 0),
                                         stop=(q == 3))
                    osb = kp.tile([128, SLAB], f32, tag="osb", name="osb")
                    nc.scalar.copy(osb[:], ps_o[:])
                    nc.sync.dma_start(out_e[ts(ct), tok], osb[:])
                xs = xs_next
                lam_bc = lam_bc_next
                trig = trig_next
    nc.compile()
    return nc


_CACHE = {}


def _host_inputs(hidden_states, q_w, k_w, v_w, g_w, s_w, sg_w, gn_w, o_w):
    half = HALF
    inv = 1.0 / (ROPE_BASE ** (np.arange(half, dtype=np.float64) / half))
    ang = np.arange(T, dtype=np.float64)[None, :] * inv[:, None]   # [half, T]
    cosT = np.cos(ang).astype(np.float32)
    sinT = np.sin(ang).astype(np.float32)
    jj, tt_ = np.meshgrid(np.arange(128), np.arange(128), indexing="ij")
    tril = (jj <= tt_).astype(np.float32)
    ident = np.eye(128, dtype=np.float32)
    ones = np.ones((1, 128), np.float32)

    b16 = ml_dtypes.bfloat16
    in_maps = []
    for c in range(NCORE):
        b, hg = c // 2, c % 2
        cs = slice(hg * COLS, (hg + 1) * COLS)
        gn_ext = np.asarray(gn_w, np.float32)[np.arange(COLS) % DV]
        in_maps.append({
            "xT": np.ascontiguousarray(np.asarray(hidden_states[b], np.float32).T).astype(b16),
            "wq": np.ascontiguousarray(np.asarray(q_w, np.float32)[:, cs]).astype(b16),
            "wk": np.ascontiguousarray(np.asarray(k_w, np.float32)[:, cs]).astype(b16),
            "ws": np.ascontiguousarray(np.asarray(s_w, np.float32)[:, cs]).astype(b16),
            "wv": np.ascontiguousarray(np.asarray(v_w, np.float32)[:, cs]).astype(b16),
            "wg": np.ascontiguousarray(np.asarray(g_w, np.float32)[:, cs]).astype(b16),
            "wsg": np.ascontiguousarray(np.asarray(sg_w, np.float32)[:, 2 * hg:2 * hg + 2]).astype(b16),
            "wo": (np.asarray(o_w, np.float32)[cs, :]
                   * gn_ext[:, None]).astype(b16),
            "cosT": cosT, "sinT": sinT,
            "trilq": tril * np.float32(QSCALE), "tril1": tril,
            "tril1b": tril.astype(b16), "identb": ident.astype(b16),
            "ident": ident, "onesc": ones.astype(b16),
            "onek": np.ones((128, 1), b16),
            "zeros": np.zeros((128, 512), b16),
        })
    return in_maps


def kernel(**inputs):
    from concourse.bass_utils import run_bass_kernel_spmd
    if "nc" not in _CACHE:
        _CACHE["nc"] = build()
    nc = _CACHE["nc"]
    in_maps = _host_inputs(**inputs)
    r = run_bass_kernel_spmd(nc, in_maps, core_ids=list(range(NCORE)))
    out = np.empty((B, T, D), np.float32)
    for b in range(B):
        out[b] = (r.results[2 * b]["outT"] + r.results[2 * b + 1]["outT"]).T
    return out

